# revision 1
# baseline (speedup 1.0000x reference)
"""Trainium2 Bass kernel for a 16-head MHA block (B=1, S=4096, H=1024).

Sharding: tensor-parallel over heads — each of the 8 cores owns 2 heads
(128 of the 1024 Wq/Wk/Wv output channels) and computes 512 rows of the
final (scrambled) output; the host concatenates the row blocks.

Per-core dataflow (all matmuls in bf16, fp32 PSUM accumulation):
  qT/kT = relu(W @ x.T + b)      layout [128 chan, 4096 seq]   (transposed)
  v     = relu(x @ W.T + b)      layout [4096 seq, 128 chan]   (natural)
  S_T[t,s]  = sum_d kT[d,t] qT[d,s]          (2 heads row-packed, K=64)
  E = exp(S_T/8)   split across two engines BY HEAD, each head's scores
      in its own single-bank PSUM tile so the engines never touch the
      same PSUM bank (Tile serializes cross-engine same-bank access):
      head0: VectorE Schraudolph bit-trick (int16 -> bf16 bits)
      head1: ScalarE exact Exp
  num/den   = sum_t [v|1][t,d'] E[t,s]       (M=65 matmul: row 64 = denom)
  epilogue: den copy on ScalarE, recip + normalize + residual on DVE,
  partition-broadcast on GpSimd, all interleaved as side-tasks into the
  next s-block's iteration loop.
"""

import math

import numpy as np
import ml_dtypes

import concourse.bass as bass
import concourse.tile as tile
from concourse import bacc, mybir
from concourse.bass import ds, ts
from concourse.bass_utils import run_bass_kernel_spmd

BF16 = ml_dtypes.bfloat16
S = 4096
H = 1024
NCORES = 8
OC = H // NCORES  # 128 output channels (2 heads) per core
SBLK = 512  # s-block width
NSB = S // SBLK  # 8
NT = S // 128  # 32 t-chunks
NKC = H // 128  # 8 contraction chunks for projections

# Schraudolph exp for the DVE share: bf16 bits of exp(s/8) are approx
# round(s * (16/ln2) + 128*(127 - c)); scores are >= 0 (post-relu q,k).
SCH_MUL = 16.0 / math.log(2.0)
SCH_ADD = 128.0 * (127.0 - 0.0437)

_CACHE = {}


def _build_nc():
    f32 = mybir.dt.float32
    bf16 = mybir.dt.bfloat16
    i16 = mybir.dt.int16
    add = mybir.AluOpType.add
    mult = mybir.AluOpType.mult
    amax = mybir.AluOpType.max
    Exp = mybir.ActivationFunctionType.Exp
    Relu = mybir.ActivationFunctionType.Relu

    nc = bacc.Bacc("TRN2", target_bir_lowering=False, debug=False)

    # inputs are host-pre-arranged so every DMA is contiguous per partition:
    # x*: [sb, p, c, s_local], w*: [p, c, o]
    xq_r = nc.dram_tensor(
        "xq", [NSB, 128, NKC, SBLK], bf16, kind="ExternalInput"
    ).ap()
    xk_r = nc.dram_tensor(
        "xk", [NSB, 128, NKC, SBLK], bf16, kind="ExternalInput"
    ).ap()
    xv_r = nc.dram_tensor(
        "xv", [NSB, 128, NKC, SBLK], bf16, kind="ExternalInput"
    ).ap()
    wq_r = nc.dram_tensor("wq", [128, NKC, OC], bf16, kind="ExternalInput").ap()
    wk_r = nc.dram_tensor("wk", [128, NKC, OC], bf16, kind="ExternalInput").ap()
    wv_r = nc.dram_tensor("wv", [128, NKC, OC], bf16, kind="ExternalInput").ap()
    bq = nc.dram_tensor("bq", [OC, 1], f32, kind="ExternalInput").ap()
    bk = nc.dram_tensor("bk", [OC, 1], f32, kind="ExternalInput").ap()
    bv = nc.dram_tensor("bv", [1, OC], bf16, kind="ExternalInput").ap()
    qres = nc.dram_tensor("qres", [512, H], bf16, kind="ExternalInput").ap()
    out = nc.dram_tensor("out", [512, H], bf16, kind="ExternalOutput").ap()
    # residual/output rows: local row = 256*hl + 4*d + j
    qres_r = qres.rearrange("(hl d j) m -> hl d j m", hl=2, d=64)
    out_r = out.rearrange("(hl d j) m -> hl d j m", hl=2, d=64)

    with tile.TileContext(nc) as tc:
        with (
            tc.tile_pool(name="const", bufs=1) as constp,
            tc.tile_pool(name="persist", bufs=1) as persist,
            tc.tile_pool(name="stage", bufs=2) as stage,
            tc.tile_pool(name="exps", bufs=8) as expp,
            tc.tile_pool(name="epi", bufs=2) as epi,
            tc.tile_pool(name="ps_d", bufs=2, space="PSUM") as ps_d,
            tc.tile_pool(name="ps_s", bufs=2, space="PSUM") as ps_s,
            tc.tile_pool(name="ps_av", bufs=2, space="PSUM") as ps_av,
        ):
            # ---- constants ----
            wq_sb = constp.tile([128, NKC, OC], bf16)
            wk_sb = constp.tile([128, NKC, OC], bf16)
            wv_sb = constp.tile([128, NKC, OC], bf16)
            nc.sync.dma_start(wk_sb[:], wk_r)
            nc.sync.dma_start(wv_sb[:], wv_r)
            nc.sync.dma_start(wq_sb[:], wq_r)
            bq_sb = constp.tile([OC, 1], f32)
            bk_sb = constp.tile([OC, 1], f32)
            bv_sb = constp.tile([1, OC], bf16)
            nc.scalar.dma_start(bq_sb[:], bq)
            nc.scalar.dma_start(bk_sb[:], bk)
            nc.scalar.dma_start(bv_sb[:], bv)
            ones_rowb = constp.tile([1, 128], bf16)
            nc.vector.memset(ones_rowb[:], 1.0)

            qT_sb = persist.tile([128, S], bf16)
            kT_sb = persist.tile([128, S], bf16)
            # per (ti, head): 128 cols = [v[t, d0..d63] | ones | pad63]
            # (block stride 256B: dma_start_transpose needs aligned dst)
            v_sb = persist.tile([128, NT * 2 * 128], bf16)
            v_sb_r = v_sb.rearrange("p (t h w) -> p t h w", t=NT, h=2)
            nc.vector.memset(v_sb_r[:, :, :, 64:65], 1.0)

            # ---- helper defs ----
            side_sched = []  # [slot, fn]: fn runs at first iteration >= slot

            def q_proj(sb):
                ss = ds(sb * SBLK, SBLK)
                xq_st = stage.tile([128, NKC, SBLK], bf16, name="xq_st")
                nc.sync.dma_start(xq_st[:], xq_r[sb])
                qp = ps_av.tile([128, 1024], f32, tag="av", name="qp")
                for ci in range(NKC):
                    nc.tensor.matmul(
                        qp[:, :SBLK], wq_sb[:, ci, :], xq_st[:, ci, :],
                        start=(ci == 0), stop=(ci == NKC - 1),
                    )
                nc.scalar.activation(
                    qT_sb[:, ss], qp[:, :SBLK], Relu, bias=bq_sb[:]
                )

            def q_proj_sched(sb, slots):
                # split into matmul halves + relu, spaced so nothing waits
                # at its engine-queue head
                ss = ds(sb * SBLK, SBLK)
                xq_st = stage.tile([128, NKC, SBLK], bf16, name="xq_st")
                nc.sync.dma_start(xq_st[:], xq_r[sb])
                st = {}

                def mm(c0, c1):
                    def f():
                        if "qp" not in st:
                            st["qp"] = ps_av.tile(
                                [128, 1024], f32, tag="av", name="qp"
                            )
                        for ci in range(c0, c1):
                            nc.tensor.matmul(
                                st["qp"][:, :SBLK], wq_sb[:, ci, :],
                                xq_st[:, ci, :],
                                start=(ci == 0), stop=(ci == NKC - 1),
                            )
                    return f

                def rl():
                    nc.scalar.activation(
                        qT_sb[:, ss], st["qp"][:, :SBLK], Relu, bias=bq_sb[:]
                    )

                side_sched.extend(
                    [[slots[0], mm(0, 4)], [slots[1], mm(4, 8)], [slots[2], rl]]
                )

            def scores_exp(sb, ti):
                ss = ds(sb * SBLK, SBLK)
                tt = ds(ti * 128, 128)
                scd = ps_d.tile([128, 512], f32, tag="d", name="scd")
                scs = ps_s.tile([128, 512], f32, tag="s", name="scs")
                nc.tensor.matmul(
                    scd[:], kT_sb[0:64, tt], qT_sb[0:64, ss],
                    start=True, stop=True,
                )
                nc.tensor.matmul(
                    scs[:], kT_sb[64:128, tt], qT_sb[64:128, ss],
                    start=True, stop=True,
                )
                ex = expp.tile([128, 1024], bf16, name="ex")
                nc.vector.tensor_scalar(
                    ex[:, 0:512].bitcast(i16), scd[:],
                    SCH_MUL, SCH_ADD, mult, add,
                )
                nc.scalar.activation(
                    ex[:, 512:1024], scs[:], Exp, scale=0.125
                )
                return ex

            def av_mm(av, ti, ex):
                for hl in range(2):
                    nc.tensor.matmul(
                        av[0:65, ts(hl, SBLK)],
                        v_sb_r[:, ti, hl, 0:65],
                        ex[:, ts(hl, SBLK)],
                        start=(ti == 0), stop=(ti == NT - 1),
                    )

            pend = []  # up to 2 deferred (ti, ex) AV inputs

            def attn_ti(sb, av, ti):
                ex = scores_exp(sb, ti)
                # keep AV two iterations behind the scores so the AV
                # matmuls never wait on a freshly-produced ex (which would
                # serialize exp -> AV -> next scores -> next exp)
                if len(pend) >= 2:
                    pti, pex = pend.pop(0)
                    av_mm(av, pti, pex)
                pend.append((ti, ex))
                if ti == NT - 1:
                    for pti, pex in pend:
                        av_mm(av, pti, pex)
                    pend.clear()
                # slot-scheduled side work (epilogue pieces, next q_proj),
                # spaced so each op's inputs are ready before it reaches
                # its engine-queue head
                side_sched.sort(key=lambda x: x[0])
                while side_sched and ti >= side_sched[0][0]:
                    side_sched.pop(0)[1]()

            def epilogue(sb, av):
                # normalize + residual + store; split into side-tasks that
                # interleave with the next s-block's iterations
                j = sb // 2
                mm = ds((sb % 2) * SBLK, SBLK)
                qrt = epi.tile([64, 1024], bf16, name="qrt")
                for hl in range(2):
                    nc.sync.dma_start(
                        qrt[:, ts(hl, SBLK)], qres_r[hl, :, j, mm]
                    )
                d64 = epi.tile([65, 1024], f32, name="d64")
                den0 = epi.tile([1, 1024], f32, name="den0")
                bcd = epi.tile([64, 1024], f32, name="bcd")
                bcs = epi.tile([64, 1024], f32, name="bcs")
                prod = epi.tile([64, 1024], bf16, name="prod")
                outt = epi.tile([64, 1024], bf16, name="outt")

                def dn():
                    # den row (partition 64) -> SBUF -> partition 0 -> bcast
                    nc.scalar.copy(d64[64:65, :], av[64:65, :])
                    nc.sync.dma_start(den0[:], d64[64:65, :])
                    nc.gpsimd.partition_broadcast(bcd[:], den0[:])

                def rc():
                    nc.vector.reciprocal_approx_fast(bcs[:], bcd[:])

                def pr():
                    nc.vector.tensor_tensor(
                        prod[:], av[0:64, :], bcs[:], mult
                    )

                def fin():
                    nc.vector.tensor_tensor(outt[:], prod[:], qrt[:], add)
                    for hl in range(2):
                        nc.sync.dma_start(
                            out_r[hl, :, j, mm], outt[:, ts(hl, SBLK)]
                        )

                side_sched.extend([[2, dn], [6, rc], [7, pr], [9, fin]])

            # ---- k/v projections interleaved with attention(0) ----
            av0 = ps_av.tile([128, 1024], f32, name="av")
            for sb in range(NSB):
                ss = ds(sb * SBLK, SBLK)
                xk_st = stage.tile([128, NKC, SBLK], bf16, name="xk_st")
                xv_st = stage.tile([128, NKC, SBLK], bf16, name="xv_st")
                if sb == 0:
                    nc.sync.dma_start(xk_st[:, 0:2, :], xk_r[sb, :, 0:2, :])
                    nc.sync.dma_start(xk_st[:, 2:NKC, :], xk_r[sb, :, 2:NKC, :])
                else:
                    nc.sync.dma_start(xk_st[:], xk_r[sb])
                nc.scalar.dma_start(xv_st[:], xv_r[sb])

                kp = ps_s.tile([128, 512], f32, tag="s", name="kp")
                for ci in range(NKC):
                    nc.tensor.matmul(
                        kp[:], wk_sb[:, ci, :], xk_st[:, ci, :],
                        start=(ci == 0), stop=(ci == NKC - 1),
                    )
                nc.scalar.activation(
                    kT_sb[:, ss], kp[:], Relu, bias=bk_sb[:]
                )
                for tj in range(4):
                    ti = sb * 4 + tj
                    vp = ps_d.tile([128, 512], f32, tag="d", name="vp")
                    for ci in range(NKC):
                        nc.tensor.matmul(
                            vp[:, 0:128], xv_st[:, ci, ts(tj, 128)], wv_sb[:, ci, :],
                            start=(ci == 0), stop=False,
                        )
                    nc.tensor.matmul(
                        vp[:, 0:128], ones_rowb[:1, :], bv_sb[:1, :],
                        start=False, stop=True,
                    )
                    nc.vector.tensor_scalar_max(
                        v_sb_r[:, ti, :, 0:64],
                        vp[:, 0:128].rearrange("p (h w) -> p h w", h=2),
                        0.0,
                    )
                    if sb == 0 and tj == 0:
                        q_proj(0)
                    attn_ti(0, av0, ti)
            epilogue(0, av0)

            # ---- remaining attention s-blocks ----
            q_proj(1)
            for sb in range(1, NSB):
                if sb + 1 < NSB:
                    q_proj_sched(sb + 1, (8, 10, 12))
                av = ps_av.tile([128, 1024], f32, name="av")
                for ti in range(NT):
                    attn_ti(sb, av, ti)
                epilogue(sb, av)
            for _, fn in sorted(side_sched, key=lambda x: x[0]):
                fn()
            side_sched.clear()

    nc.compile()
    return nc


def _get_nc():
    if "nc" not in _CACHE:
        _CACHE["nc"] = _build_nc()
    return _CACHE["nc"]


def _arr_x(x2):
    # [S, H] -> [sb, p, c, s_local] with H = c*128 + p, S = sb*512 + s
    xT = x2.T.astype(BF16)  # [H, S]
    return np.ascontiguousarray(
        xT.reshape(NKC, 128, NSB, SBLK).transpose(2, 1, 0, 3)
    )


def _arr_w(wT):
    # [H, OC] -> [p, c, o]
    return np.ascontiguousarray(
        wT.astype(BF16).reshape(NKC, 128, OC).transpose(1, 0, 2)
    )


def kernel(queries, keys, values, Wq_w, Wq_b, Wk_w, Wk_b, Wv_w, Wv_b, **kw):
    nc = _get_nc()
    q2 = np.asarray(queries, np.float32).reshape(S, H)
    k2 = np.asarray(keys, np.float32).reshape(S, H)
    v2 = np.asarray(values, np.float32).reshape(S, H)
    xq4 = _arr_x(q2)
    xk4 = _arr_x(k2)
    xv4 = _arr_x(v2)

    in_maps = []
    for c in range(NCORES):
        o = slice(OC * c, OC * (c + 1))
        in_maps.append(
            {
                "xq": xq4,
                "xk": xk4,
                "xv": xv4,
                "wq": _arr_w(np.asarray(Wq_w)[o].T),
                "wk": _arr_w(np.asarray(Wk_w)[o].T),
                "wv": _arr_w(np.asarray(Wv_w)[o].T),
                "bq": np.asarray(Wq_b, np.float32)[o].reshape(OC, 1),
                "bk": np.asarray(Wk_b, np.float32)[o].reshape(OC, 1),
                "bv": np.asarray(Wv_b)[o].astype(BF16).reshape(1, OC),
                "qres": np.ascontiguousarray(
                    q2[512 * c : 512 * (c + 1)]
                ).astype(BF16),
            }
        )

    res = run_bass_kernel_spmd(
        nc, in_maps, list(range(NCORES)), **_CACHE.get("run_kwargs", {})
    )
    _CACHE["last_results"] = res
    full = np.concatenate(
        [np.asarray(res.results[c]["out"], np.float32) for c in range(NCORES)],
        axis=0,
    )
    return full.reshape(1, S, H)

